# revision 40
# baseline (speedup 1.0000x reference)
"""EntityEncoder Trainium2 kernel (v2).

Computes, for each (batch, sentence j): ragged per-entity span mean-pool over
token embeddings, then a Linear projection:

    pooled[b, j, k, :] = mean(zipped_entity[b, j, start_kj:end_kj, :])
    out[b, j*K+k, :]   = pooled @ W + b

Strategy (8 NeuronCores, memory-bound):
  - Only span-member tokens are ever touched: rows outside the K spans
    (token 0 and the separator tokens themselves) are dropped at host-pack
    time. All 128 sentences are sorted by needed-row count and dealt
    rank-sliced: slot s (0..15) holds ranks [8s, 8s+8), one per core, so the
    shared NEFF's per-slot capacity (max of 8 nearly-equal counts) wastes ~2%.
    The NEFF is compile-time specialized on the 16 capacities (cached).
  - The z stream is fp8 e3m4 (4 mantissa bits; |z|<15.5 in range), quantized
    host-side with error diffusion along each sentence's packed rows so each
    span-sum's quantization error telescopes to ~1 quantum instead of
    sqrt(n) quanta; measured end-to-end rel err ~6e-3 (vs 4e-4 for f16) for
    half the HBM traffic. Host packs [128, C, 768] chunk-transposed; DMA
    reads exact rows in pieces alternating the two HWDGE rings (sync+ACT),
    all issued up front (no compute ever queues on those rings mid-stream).
  - Pooling matmuls put Z as the STATIONARY operand (full 128-column weight
    loads get the 4-elem/cycle fast-weight-load path) and the 0/1 span masks
    as the moving operand: psum[d, e] += z_chunk[t, d]^T @ mask[t, e]. The
    pooled sums land already TRANSPOSED [d on partitions, entities free], so
    the old tail transposes disappear. Masks are compact MW-wide slabs per
    (chunk, entity-block), host-built.
  - Per 32-entity block, as soon as its last chunk is pooled: PSUM->SBUF f16
    copy (DVE, + ACT for the final block once the z rings are drained), then
    a W-stationary Linear (12 matmuls, FWL) accumulating into po[o, e], with
    the bias folded in as a rank-1 seed matmul b[o] (x) counts[e] emitted
    start=True. A single fused DVE multiply by 1/count then writes the f16
    output tile: (sums@W)*rc + b*(count*rc) reproduces the reference
    exactly, including 0/0 -> NaN (po=0, rc=inf -> NaN) and negative-count
    spans. Block tails are emitted LAG chunks after their last pool matmul
    so the in-order PE queue never stalls pooling behind a tail dependency;
    block 0's whole tail hides under the tail half of the z stream.
"""

import os
import numpy as np
from contextlib import ExitStack

BS, J, L, D = 32, 4, 512, 768
K = 4
OUT = 256
NCORES = 8
NSENT = BS * J           # 128 sentence tasks
NSLOT = NSENT // NCORES  # 16 slots per core
NG = int(os.environ.get("BASSK_NG", "1"))   # sentence groups per core
GS = NSLOT // NG         # slots (sentences) per group
NE = GS * K              # entities per group
DC = D // 128            # 6 D-chunks
HO = OUT // 128          # 2 output halves

# z/mask operand dtype: "f8e3" (default), "f8e4", "f16"
ZDT_MODE = os.environ.get("BASSK_ZDT", "f8e3")
# error-diffusion quantization of the z stream
DIFFUSE = os.environ.get("BASSK_DIFFUSE", "1") == "1"
# mask slab width (entity block granularity of the tail pipeline)
MB = int(os.environ.get("BASSK_MB", "32"))
MW = min(MB, NE)
NBLK = max(1, NE // MB)
# interior z piece boundaries (chunk units, scaled to ctot=16); "auto" =
# structural: small first piece, a cut right after each non-final block's
# last chunk, small final piece
ZBOUNDS = os.environ.get("BASSK_ZBOUNDS", "auto")
# chunks between a block's last pool matmul and its tail emission
LAG = int(os.environ.get("BASSK_LAG", "3"))
# issue the final piece early on the scalar ring and pool it first, so the
# stream's last-byte -> last-pool chain ends on an earlier piece
SPLITLAST = os.environ.get("BASSK_SPLITLAST", "1") == "1"

_CACHE = {}


def _z_dt(mybir):
    return {
        "f8e3": mybir.dt.float8e3,
        "f8e4": mybir.dt.float8e4,
        "f16": mybir.dt.float16,
    }[ZDT_MODE]


def _z_np():
    import ml_dtypes

    return {
        "f8e3": np.dtype(ml_dtypes.float8_e3m4),
        "f8e4": np.dtype(ml_dtypes.float8_e4m3),
        "f16": np.dtype(np.float16),
    }[ZDT_MODE]


def _geom(caps):
    """Per-group geometry from the 16 slot capacities: token totals, chunk
    counts, chunk offsets, group processing order (ascending size)."""
    caps = tuple(int(c) for c in caps)
    Ts = [sum(caps[g * GS : (g + 1) * GS]) for g in range(NG)]
    Cs = [(t + 127) // 128 for t in Ts]
    cum = np.cumsum([0] + Cs)
    co = [int(c) for c in cum[:-1]]
    ctot = int(cum[-1])
    gorder = sorted(range(NG), key=lambda g: Ts[g])
    return Ts, Cs, co, ctot, gorder


def _blocks(caps):
    """Entity-block coverage per (group, chunk): which MW-wide entity blocks
    the chunk's tokens touch, with the chunk-local slab slot of each."""
    Ts, Cs, co, ctot, _ = _geom(caps)
    chunk_blocks = {}            # (g, c) -> list of (block, local_slab_idx)
    for g in range(NG):
        offs = np.cumsum([0] + [caps[g * GS + j] for j in range(GS)])
        for c in range(Cs[g]):
            lo, hi = c * 128, min((c + 1) * 128, Ts[g])
            s0 = int(np.searchsorted(offs, lo, side="right")) - 1
            s1 = int(np.searchsorted(offs, hi - 1, side="right")) - 1
            blks = (
                sorted(set((K * s) // MW for s in range(s0, s1 + 1)))
                if NBLK > 1
                else [0]
            )
            chunk_blocks[(g, c)] = [(b, si) for si, b in enumerate(blks)]
    return chunk_blocks


def _piece_bounds(caps):
    """Chunk-piece boundaries of the fused stream (global chunk units). Each
    dma_start costs ~0.5us of serialized HWDGE descriptor generation, so few
    pieces; but a piece only becomes poolable when its WHOLE data lands
    (+~0.9us completion latency), so: small first piece (PE starts early), a
    cut right after each non-final entity block's last chunk (its tail chain
    starts as early as possible), small final piece."""
    Ts, Cs, co, ctot, _ = _geom(caps)
    cuts = {0, ctot}
    if ZBOUNDS != "auto":
        for b in ZBOUNDS.split(","):
            if b.strip():
                bi = (int(b) * ctot + 8) // 16  # scale the plan to this ctot
                if 0 < bi < ctot:
                    cuts.add(bi)
        return sorted(cuts)
    chunk_blocks = _blocks(caps)
    lc = {}
    for (g, c), lst in chunk_blocks.items():
        for bidx, _ in lst:
            lc[(g, bidx)] = max(lc.get((g, bidx), -1), co[g] + c)
    cuts.add(min(2, ctot))
    for (g, bidx), v in lc.items():
        if bidx < NBLK - 1 and 0 < v + 1 < ctot:
            cuts.add(v + 1)
    if ctot >= 4:
        cuts.add(ctot - 2)
    return sorted(cuts)


def _stream_layout(caps):
    """Fused single-stream layout: every input rides ONE dram buffer
    [128, SROW] of z-dtype bytes.

    Per partition row: ctot chunk records of RP bytes — 768 B of z followed
    by SLABMAX 32 B mask slabs (a chunk's masks arrive with its data) — with
    a section block (W as f16, rcount as f32, counts+bias as f16, bitcast
    views) spliced in before chunk SEC_CH so it rides the middle DMA piece.
    """
    Ts, Cs, co, ctot, _ = _geom(caps)
    chunk_blocks = _blocks(caps)
    slabmax = max(1, max(len(v) for v in chunk_blocks.values()))
    rp = D + slabmax * MW
    bounds = _piece_bounds(caps)
    sec_ch = bounds[1] if len(bounds) > 2 else 0
    secb = DC * OUT * 2 + NG * NE * 4 + (NG * NE + OUT) * 2
    sec0 = sec_ch * rp
    woff, rcoff, cboff = sec0, sec0 + DC * OUT * 2, sec0 + DC * OUT * 2 + NG * NE * 4
    srow = ctot * rp + secb

    def zoff(gc):
        return gc * rp + (secb if gc >= sec_ch else 0)

    def colbound(gc):
        return gc * rp + (secb if gc > sec_ch else 0)

    return {
        "rp": rp, "slabmax": slabmax, "bounds": bounds, "sec_ch": sec_ch,
        "secb": secb, "woff": woff, "rcoff": rcoff, "cboff": cboff,
        "srow": srow, "zoff": zoff, "colbound": colbound,
    }


def _build_nc(niter=1, hw_loop=0, nch_sj=None):
    """nch_sj: the 16-tuple of slot row capacities (compile-time plan)."""
    import contextlib

    import concourse.bass as bass
    import concourse.mybir as mybir
    from concourse.bacc import Bacc
    from concourse.tile import TileContext

    f32, f16 = mybir.dt.float32, mybir.dt.float16
    zdt = _z_dt(mybir)
    caps = nch_sj if nch_sj is not None else tuple(L for _ in range(NSLOT))
    Ts, Cs, co, ctot, gorder = _geom(caps)
    chunk_blocks = _blocks(caps)
    lay = _stream_layout(caps)
    RP, SROW, zoff, colbound = lay["rp"], lay["srow"], lay["zoff"], lay["colbound"]

    nc = Bacc(trn_type="TRN2")
    sb = nc.declare_dram_parameter("sb", [128, SROW], zdt, isOutput=False)
    out = nc.declare_dram_parameter("out", [NG, 128, HO, NE], f16, isOutput=True)

    with TileContext(nc) as tc:
        with ExitStack() as ctx:
            zpool = ctx.enter_context(tc.tile_pool(name="zp", bufs=min(niter, 2)))
            ptpool = ctx.enter_context(tc.tile_pool(name="pt", bufs=2))
            otpool = ctx.enter_context(tc.tile_pool(name="otp", bufs=2))
            psum_ps = ctx.enter_context(tc.tile_pool(name="ps", bufs=2, space="PSUM"))
            psum_po = ctx.enter_context(tc.tile_pool(name="po", bufs=2, space="PSUM"))

            loop_cm = tc.For_i(0, hw_loop, 1) if hw_loop else contextlib.nullcontext()
            with loop_cm:
              for it in range(niter):
                # The ENTIRE input (z chunks with interleaved mask slabs, plus
                # the W/rc/cnt/bias section riding the middle piece) streams
                # through a few dma_starts on the sync ring only: the HWDGE
                # descriptor generator is a single shared resource (~0.5us per
                # dma_start, serialized), so fewer+bigger pieces win.
                st = zpool.tile([128, SROW], zdt, name=f"st{it}", tag="st")
                pieces = list(zip(lay["bounds"][:-1], lay["bounds"][1:]))
                split = SPLITLAST and NG == 1 and len(pieces) >= 3
                order_pieces = (
                    [(pieces[0], nc.sync), (pieces[-1], nc.scalar)]
                    + [(p, nc.sync) for p in pieces[1:-1]]
                    if split
                    else [(p, nc.sync) for p in pieces]
                )
                for (a, bhi), eng in order_pieces:
                    eng.dma_start(
                        out=st[:, colbound(a) : colbound(bhi)],
                        in_=sb[:, colbound(a) : colbound(bhi)],
                    )
                w_t = st[:, lay["woff"] : lay["woff"] + DC * OUT * 2].bitcast(f16)
                rc_t = st[:, lay["rcoff"] : lay["rcoff"] + NG * NE * 4].bitcast(f32)
                cb_t = st[
                    0:1, lay["cboff"] : lay["cboff"] + (NG * NE + OUT) * 2
                ].bitcast(f16)
                cnt_t = cb_t[:, 0 : NG * NE]
                b_t = cb_t[:, NG * NE : NG * NE + OUT]

                for gi, g in enumerate(gorder):
                    T, C, CO = Ts[g], Cs[g], co[g]
                    last_g = gi == NG - 1
                    cb = {c: chunk_blocks[(g, c)] for c in range(C)}
                    # pool chunks in DATA-ARRIVAL order (final piece first
                    # when it rides the early scalar-ring DMA)
                    if split:
                        pool_order = (
                            list(range(*pieces[0]))
                            + list(range(*pieces[-1]))
                            + [c for a, b_ in pieces[1:-1] for c in range(a, b_)]
                        )
                    else:
                        pool_order = list(range(C))
                    pos = {c: i for i, c in enumerate(pool_order)}
                    lc = {}  # block -> emission position of its last chunk
                    for c in range(C):
                        for bidx, _ in cb[c]:
                            lc[bidx] = max(lc.get(bidx, -1), pos[c])

                    ps = psum_ps.tile([128, DC, NE], f32, name=f"ps{it}_{g}", tag="ps")
                    po = psum_po.tile([128, HO, NE], f32, name=f"po{it}_{g}", tag="po")
                    pt = ptpool.tile([128, DC, NE], f16, name=f"pt{it}_{g}", tag="pt")
                    ot = otpool.tile([128, HO, NE], f16, name=f"ot{it}_{g}", tag="ot")

                    def emit_copy(bidx, last_b):
                        # One DVE op; ACT is avoided entirely (any ACT op
                        # pulls a 1.3us activation-table load into the tail).
                        sl = slice(bidx * MW, (bidx + 1) * MW)
                        nc.vector.tensor_copy(pt[:, :, sl], ps[:, :, sl])

                    # PSUM start/stop semantics: start_tensor_calc pending-
                    # zeroes the ENTIRE 2KB bank (ZERO_REGION), and only one
                    # accumulation group may be open per bank. So each bank
                    # gets start=True on its very FIRST matmul and stop=True
                    # on its very LAST; every other write relies on the
                    # store-on-first-touch pending-zero semantics.
                    po_state = {"first": True}

                    def emit_linear(bidx):
                        sl = slice(bidx * MW, (bidx + 1) * MW)
                        cs = slice(g * NE + bidx * MW, g * NE + (bidx + 1) * MW)
                        # bias (x) counts seeds this block's po columns
                        for h in range(HO):
                            nc.tensor.matmul(
                                po[:, h, sl],
                                lhsT=b_t[0:1, h * 128 : (h + 1) * 128],
                                rhs=cnt_t[0:1, cs],
                                start=po_state["first"],
                                stop=False,
                            )
                            po_state["first"] = False
                        for h in range(HO):
                            for dc in range(DC):
                                nc.tensor.matmul(
                                    po[:, h, sl],
                                    lhsT=w_t[:, dc * OUT + h * 128 : dc * OUT + (h + 1) * 128],
                                    rhs=pt[:, dc, sl],
                                    start=False,
                                    stop=(
                                        bidx == NBLK - 1
                                        and h == HO - 1
                                        and dc == DC - 1
                                    ),
                                )

                    def emit_scale(bidx, last_b):
                        # All scales are emitted AFTER every copy: DVE is
                        # in-order, so a scale waiting on PE must never sit
                        # ahead of a later block's PSUM->SBUF copies.
                        sl = slice(bidx * MW, (bidx + 1) * MW)
                        cs = slice(g * NE + bidx * MW, g * NE + (bidx + 1) * MW)
                        rc_b = rc_t[:, cs].unsqueeze(1).broadcast_to([128, HO, MW])
                        nc.vector.tensor_mul(ot[:, :, sl], po[:, :, sl], rc_b)
                        if last_b:
                            # one out DMA for the whole group, on sync (its
                            # ring is idle once the input pieces are issued)
                            nc.sync.dma_start(out=out[g, :, :, :], in_=ot[:, :, :])

                    tail_due = {}
                    done_lin = set()
                    for bidx in range(NBLK - 1):  # last block's linear: post-loop
                        if lc[bidx] + LAG < C:
                            tail_due.setdefault(lc[bidx] + LAG, []).append(bidx)

                    ps_first = True
                    for ci, c in enumerate(pool_order):
                        zo = zoff(CO + c)
                        for blk in range(DC):
                            for i, (bidx, si) in enumerate(cb[c]):
                                nc.tensor.matmul(
                                    ps[:, blk, bidx * MW : (bidx + 1) * MW],
                                    lhsT=st[:, zo + blk * 128 : zo + (blk + 1) * 128],
                                    rhs=st[
                                        :, zo + D + si * MW : zo + D + (si + 1) * MW
                                    ],
                                    start=ps_first,
                                    stop=(
                                        ci == C - 1
                                        and blk == DC - 1
                                        and i == len(cb[c]) - 1
                                    ),
                                )
                                ps_first = False
                        for bidx in range(NBLK):
                            if lc[bidx] == ci:
                                emit_copy(bidx, last_b=(last_g and bidx == NBLK - 1))
                        for bidx in tail_due.get(ci, []):
                            emit_linear(bidx)
                            done_lin.add(bidx)
                    for bidx in range(NBLK):
                        if bidx not in done_lin:
                            emit_linear(bidx)
                    for bidx in range(NBLK):
                        emit_scale(bidx, last_b=(last_g and bidx == NBLK - 1))
    nc.finalize()
    return nc


def _span_info(sep):
    sep2 = np.asarray(sep).reshape(NSENT, K)
    starts = np.concatenate([np.ones_like(sep2[:, :1]), sep2[:, :-1] + 1], axis=-1)
    ends = sep2
    counts = (ends - starts).astype(np.float32)
    return sep2, starts, ends, counts


def _plan(sep):
    """Sort sentences by needed (span-member) rows; slot s gets global ranks
    [8s, 8s+8). Returns (order, caps)."""
    _, starts, ends, _ = _span_info(sep)
    R = np.clip(np.clip(ends - starts, 0, None).sum(-1), 1, L).astype(int)
    order = np.argsort(-R, kind="stable")
    caps = tuple(int(R[order[s * NCORES]]) for s in range(NSLOT))
    return order, caps


def _prep_in_maps(z, sep, Wf, bf, assign=None):
    order = assign if assign is not None else _plan(sep)[0]
    _, caps = _plan(sep)
    Ts, Cs, co, ctot, _ = _geom(caps)
    chunk_blocks = _blocks(caps)
    lay = _stream_layout(caps)
    RP, SROW, zoff = lay["rp"], lay["srow"], lay["zoff"]

    _, starts, ends, counts = _span_info(sep)       # [128, K]
    with np.errstate(divide="ignore"):
        rcounts = np.float32(1.0) / counts

    zdt = _z_np()
    zflat = z.reshape(NSENT, L, D)

    # Per-sentence span-row gather (+ entity label per row).
    idxs, labs = [], []
    for sid in range(NSENT):
        seg_i, seg_l = [], []
        for k in range(K):
            s_, e_ = int(starts[sid, k]), int(ends[sid, k])
            if e_ > s_:
                seg_i.append(np.arange(s_, e_))
                seg_l.append(np.full(e_ - s_, k))
        idxs.append(np.concatenate(seg_i) if seg_i else np.zeros(0, int))
        labs.append(np.concatenate(seg_l) if seg_l else np.zeros(0, int))
    cnt_rows = np.array([len(i) for i in idxs])
    maxc = max(int(cnt_rows.max()), 1)

    G = np.zeros((NSENT, maxc, D), np.float32)
    for sid in range(NSENT):
        G[sid, : cnt_rows[sid]] = zflat[sid, idxs[sid]]
    if DIFFUSE and zdt != np.float32:
        # Error-diffusion quantization along packed rows: each span-sum's
        # quantization error telescopes to ~1 quantum.
        Gq = np.empty((NSENT, maxc, D), zdt)
        carry = np.zeros((NSENT, D), np.float32)
        for l in range(maxc):
            v = G[:, l] + carry
            q = v.astype(zdt)
            Gq[:, l] = q
            carry = v - q.astype(np.float32)
    else:
        Gq = G.astype(zdt)

    # Section bytes shared by all cores: W (f16, [128, DC*OUT] d-on-partition),
    # rc (f32), cnt+b (f16, partition 0 only).
    Wm = Wf.astype(np.float16)  # [D, OUT]
    w_part = np.ascontiguousarray(
        Wm.reshape(DC, 128, OUT).transpose(1, 0, 2)
    ).view(np.uint8).reshape(128, DC * OUT * 2)
    bv = bf.astype(np.float16).reshape(OUT)

    in_maps = []
    for c in range(NCORES):
        stb = np.zeros((128, SROW), zdt)
        st8 = stb.view(np.uint8)
        rc = np.zeros((NG, NE), np.float32)
        cn = np.zeros((NG, NE), np.float16)
        for g in range(NG):
            T, C, CO = Ts[g], Cs[g], co[g]
            ztok = np.zeros((C * 128, D), zdt)
            mtok = np.zeros((C * 128, NE), zdt)
            off = 0
            for jslot in range(GS):
                s = g * GS + jslot
                sid = int(order[s * NCORES + c])
                cap = caps[s]
                n = min(int(cnt_rows[sid]), cap)
                ztok[off : off + n] = Gq[sid, :n]
                mtok[off + np.arange(n), jslot * K + labs[sid][:n]] = 1
                rc[g, jslot * K : (jslot + 1) * K] = rcounts[sid]
                cn[g, jslot * K : (jslot + 1) * K] = counts[sid]
                off += cap
            zch = ztok.reshape(C, 128, D).transpose(1, 0, 2)    # [128, C, D]
            mch = mtok.reshape(C, 128, NE).transpose(1, 0, 2)   # [128, C, NE]
            for cc in range(C):
                zo = zoff(CO + cc)
                stb[:, zo : zo + D] = zch[:, cc, :]
                for bidx, si in chunk_blocks[(g, cc)]:
                    stb[:, zo + D + si * MW : zo + D + (si + 1) * MW] = mch[
                        :, cc, bidx * MW : (bidx + 1) * MW
                    ]
        st8[:, lay["woff"] : lay["woff"] + DC * OUT * 2] = w_part
        rcb = np.broadcast_to(
            rc.reshape(1, NG * NE).view(np.uint8), (128, NG * NE * 4)
        )
        st8[:, lay["rcoff"] : lay["rcoff"] + NG * NE * 4] = rcb
        cbb = np.concatenate([cn.reshape(NG * NE), bv]).view(np.uint8)
        st8[0, lay["cboff"] : lay["cboff"] + (NG * NE + OUT) * 2] = cbb
        in_maps.append({"sb": stb})
    return in_maps


def _run(in_maps, nch_sj=None, **kwargs):
    from concourse.bass_utils import run_bass_kernel_spmd

    key = ("nc3", nch_sj, ZDT_MODE, MB, NG, ZBOUNDS, LAG)
    if key not in _CACHE:
        _CACHE[key] = _build_nc(nch_sj=nch_sj)
    return run_bass_kernel_spmd(_CACHE[key], in_maps, list(range(NCORES)), **kwargs)


def kernel(zipped_entity, entity_token_sep_idx, W, b):
    z = np.ascontiguousarray(np.asarray(zipped_entity, dtype=np.float32))
    sep = np.asarray(entity_token_sep_idx).astype(np.int64)
    Wf = np.ascontiguousarray(np.asarray(W, dtype=np.float32))
    bf = np.asarray(b, dtype=np.float32)
    assert z.shape == (BS, J, L, D) and sep.shape == (BS, J, K)

    order, caps = _plan(sep)
    res = _run(_prep_in_maps(z, sep, Wf, bf, assign=order), nch_sj=caps)
    out = np.empty((BS, J * K, OUT), np.float32)
    for c in range(NCORES):
        oc = res.results[c]["out"].astype(np.float32)  # [NG, 128, HO, NE]
        for s in range(NSLOT):
            g, jslot = divmod(s, GS)
            sid = int(order[s * NCORES + c])
            bb, jj = divmod(sid, J)
            for k in range(K):
                e = jslot * K + k
                out[bb, jj * K + k] = oc[g, :, :, e].T.reshape(OUT)
    return out


# revision 43
# speedup vs baseline: 1.0777x; 1.0777x over previous
"""EntityEncoder Trainium2 kernel (v3).

Computes, for each (batch, sentence j): ragged per-entity span mean-pool over
token embeddings, then a Linear projection:

    pooled[b, j, k, :] = mean(zipped_entity[b, j, start_kj:end_kj, :])
    out[b, j*K+k, :]   = pooled @ W + b

Strategy (8 NeuronCores, memory-bound):
  - Only span-member tokens are ever touched: rows outside the K spans
    (token 0 and the separator tokens themselves) are dropped at host-pack
    time. All 128 sentences are sorted by needed-row count and dealt
    rank-sliced: slot s (0..15) holds ranks [8s, 8s+8), one per core, so the
    shared NEFF's per-slot capacity (max of 8 nearly-equal counts) wastes ~2%.
    The NEFF is compile-time specialized on the 16 capacities (cached).
  - The z stream is fp8 e3m4 (4 mantissa bits; |z|<15.5 in range), quantized
    host-side with error diffusion along each sentence's packed rows so each
    span-sum's quantization error telescopes to ~1 quantum instead of
    sqrt(n) quanta; measured end-to-end rel err ~5e-3 (vs 4e-4 for f16) for
    half the HBM traffic.
  - ALL inputs ride ONE fused stream buffer in a handful of dma_starts on
    one ring: the HWDGE descriptor generator is a single shared resource
    (~0.5us serialized per dma_start, measured), concurrent rings/SWDGE
    degrade total DMA throughput ~20%, and each piece pays ~0.9us completion
    latency before its consumers wake. Each 128-token chunk record carries
    its own mask slabs inline (they arrive exactly with their data); W (f16),
    1/count (f32) and counts+bias (f16) live in a section spliced into the
    second piece via bitcast views. Piece boundaries: small first piece (PE
    starts early), a cut right after each non-final entity block's last
    chunk (its tail starts early), small final piece routed on the OTHER
    ring and POOLED FIRST, so the last-byte -> last-pool chain ends on an
    earlier, cheaper piece.
  - Pooling matmuls put Z as the STATIONARY operand and the 0/1 span masks
    as the moving operand: psum[d, e] += z_chunk[t, d]^T @ mask[t, e]. The
    pooled sums land already TRANSPOSED [d on partitions, entities free], so
    no tail transposes exist. PSUM start/stop: start_tensor_calc pending-
    zeroes the whole 2KB bank, so each bank gets start=True exactly once and
    stop=True on its final matmul; interior writes rely on store-on-first-
    touch semantics.
  - Per 32-entity block, when its last chunk is pooled: one DVE PSUM->SBUF
    f16 copy (never ACT: any ACT op drags a 1.3us activation-table load into
    the kernel), then a W-stationary Linear accumulating po[o, e], seeded by
    a rank-1 bias matmul b[o] (x) counts[e]. All 1/count scale-multiplies are
    emitted after every copy (the in-order DVE queue must never park a
    PE-dependent scale ahead of a later block's copies), and a single out
    DMA rides the idle sync ring at the end. (sums@W)*rc + b*(count*rc)
    reproduces the reference exactly, including 0/0 -> NaN (po=0, rc=inf ->
    NaN) and negative-count spans.
"""

import os
import numpy as np
from contextlib import ExitStack

BS, J, L, D = 32, 4, 512, 768
K = 4
OUT = 256
NCORES = 8
NSENT = BS * J           # 128 sentence tasks
NSLOT = NSENT // NCORES  # 16 slots per core
NG = int(os.environ.get("BASSK_NG", "1"))   # sentence groups per core
GS = NSLOT // NG         # slots (sentences) per group
NE = GS * K              # entities per group
DC = D // 128            # 6 D-chunks
HO = OUT // 128          # 2 output halves

# z/mask operand dtype: "f8e3" (default), "f8e4", "f16"
ZDT_MODE = os.environ.get("BASSK_ZDT", "f8e3")
# error-diffusion quantization of the z stream
DIFFUSE = os.environ.get("BASSK_DIFFUSE", "1") == "1"
# mask slab width (entity block granularity of the tail pipeline)
MB = int(os.environ.get("BASSK_MB", "32"))
MW = min(MB, NE)
NBLK = max(1, NE // MB)
# interior z piece boundaries (chunk units, scaled to ctot=16); "auto" =
# structural: small first piece, a cut right after each non-final block's
# last chunk, small final piece
ZBOUNDS = os.environ.get("BASSK_ZBOUNDS", "auto")
# chunks between a block's last pool matmul and its tail emission
LAG = int(os.environ.get("BASSK_LAG", "3"))
# issue the final piece early on the scalar ring and pool it first, so the
# stream's last-byte -> last-pool chain ends on an earlier piece
SPLITLAST = os.environ.get("BASSK_SPLITLAST", "1") == "1"

_CACHE = {}


def _z_dt(mybir):
    return {
        "f8e3": mybir.dt.float8e3,
        "f8e4": mybir.dt.float8e4,
        "f16": mybir.dt.float16,
    }[ZDT_MODE]


def _z_np():
    import ml_dtypes

    return {
        "f8e3": np.dtype(ml_dtypes.float8_e3m4),
        "f8e4": np.dtype(ml_dtypes.float8_e4m3),
        "f16": np.dtype(np.float16),
    }[ZDT_MODE]


def _geom(caps):
    """Per-group geometry from the 16 slot capacities: token totals, chunk
    counts, chunk offsets, group processing order (ascending size)."""
    caps = tuple(int(c) for c in caps)
    Ts = [sum(caps[g * GS : (g + 1) * GS]) for g in range(NG)]
    Cs = [(t + 127) // 128 for t in Ts]
    cum = np.cumsum([0] + Cs)
    co = [int(c) for c in cum[:-1]]
    ctot = int(cum[-1])
    gorder = sorted(range(NG), key=lambda g: Ts[g])
    return Ts, Cs, co, ctot, gorder


def _blocks(caps):
    """Entity-block coverage per (group, chunk): which MW-wide entity blocks
    the chunk's tokens touch, with the chunk-local slab slot of each."""
    Ts, Cs, co, ctot, _ = _geom(caps)
    chunk_blocks = {}            # (g, c) -> list of (block, local_slab_idx)
    for g in range(NG):
        offs = np.cumsum([0] + [caps[g * GS + j] for j in range(GS)])
        for c in range(Cs[g]):
            lo, hi = c * 128, min((c + 1) * 128, Ts[g])
            s0 = int(np.searchsorted(offs, lo, side="right")) - 1
            s1 = int(np.searchsorted(offs, hi - 1, side="right")) - 1
            blks = (
                sorted(set((K * s) // MW for s in range(s0, s1 + 1)))
                if NBLK > 1
                else [0]
            )
            chunk_blocks[(g, c)] = [(b, si) for si, b in enumerate(blks)]
    return chunk_blocks


def _piece_bounds(caps):
    """Chunk-piece boundaries of the fused stream (global chunk units). Each
    dma_start costs ~0.5us of serialized HWDGE descriptor generation, so few
    pieces; but a piece only becomes poolable when its WHOLE data lands
    (+~0.9us completion latency), so: small first piece (PE starts early), a
    cut right after each non-final entity block's last chunk (its tail chain
    starts as early as possible), small final piece."""
    Ts, Cs, co, ctot, _ = _geom(caps)
    cuts = {0, ctot}
    if ZBOUNDS != "auto":
        for b in ZBOUNDS.split(","):
            if b.strip():
                bi = (int(b) * ctot + 8) // 16  # scale the plan to this ctot
                if 0 < bi < ctot:
                    cuts.add(bi)
        return sorted(cuts)
    chunk_blocks = _blocks(caps)
    lc = {}
    for (g, c), lst in chunk_blocks.items():
        for bidx, _ in lst:
            lc[(g, bidx)] = max(lc.get((g, bidx), -1), co[g] + c)
    cuts.add(min(3, ctot))
    for (g, bidx), v in lc.items():
        if bidx < NBLK - 1 and 0 < v + 1 < ctot:
            cuts.add(v + 1)
    if ctot >= 4:
        cuts.add(ctot - 2)
    return sorted(cuts)


def _stream_layout(caps):
    """Fused single-stream layout: every input rides ONE dram buffer
    [128, SROW] of z-dtype bytes.

    Per partition row: ctot chunk records of RP bytes — 768 B of z followed
    by SLABMAX 32 B mask slabs (a chunk's masks arrive with its data) — with
    a section block (W as f16, rcount as f32, counts+bias as f16, bitcast
    views) spliced in before chunk SEC_CH so it rides the middle DMA piece.
    """
    Ts, Cs, co, ctot, _ = _geom(caps)
    chunk_blocks = _blocks(caps)
    slabmax = max(1, max(len(v) for v in chunk_blocks.values()))
    rp = D + slabmax * MW
    bounds = _piece_bounds(caps)
    sec_ch = bounds[1] if len(bounds) > 2 else 0
    secb = DC * OUT * 2 + NG * NE * 4 + (NG * NE + OUT) * 2
    sec0 = sec_ch * rp
    woff, rcoff, cboff = sec0, sec0 + DC * OUT * 2, sec0 + DC * OUT * 2 + NG * NE * 4
    srow = ctot * rp + secb

    def zoff(gc):
        return gc * rp + (secb if gc >= sec_ch else 0)

    def colbound(gc):
        return gc * rp + (secb if gc > sec_ch else 0)

    return {
        "rp": rp, "slabmax": slabmax, "bounds": bounds, "sec_ch": sec_ch,
        "secb": secb, "woff": woff, "rcoff": rcoff, "cboff": cboff,
        "srow": srow, "zoff": zoff, "colbound": colbound,
    }


def _build_nc(niter=1, hw_loop=0, nch_sj=None):
    """nch_sj: the 16-tuple of slot row capacities (compile-time plan)."""
    import contextlib

    import concourse.bass as bass
    import concourse.mybir as mybir
    from concourse.bacc import Bacc
    from concourse.tile import TileContext

    f32, f16 = mybir.dt.float32, mybir.dt.float16
    zdt = _z_dt(mybir)
    caps = nch_sj if nch_sj is not None else tuple(L for _ in range(NSLOT))
    Ts, Cs, co, ctot, gorder = _geom(caps)
    chunk_blocks = _blocks(caps)
    lay = _stream_layout(caps)
    RP, SROW, zoff, colbound = lay["rp"], lay["srow"], lay["zoff"], lay["colbound"]

    nc = Bacc(trn_type="TRN2")
    sb = nc.declare_dram_parameter("sb", [128, SROW], zdt, isOutput=False)
    out = nc.declare_dram_parameter("out", [NG, 128, HO, NE], f16, isOutput=True)

    with TileContext(nc) as tc:
        with ExitStack() as ctx:
            zpool = ctx.enter_context(tc.tile_pool(name="zp", bufs=min(niter, 2)))
            ptpool = ctx.enter_context(tc.tile_pool(name="pt", bufs=2))
            otpool = ctx.enter_context(tc.tile_pool(name="otp", bufs=2))
            psum_ps = ctx.enter_context(tc.tile_pool(name="ps", bufs=2, space="PSUM"))
            psum_po = ctx.enter_context(tc.tile_pool(name="po", bufs=2, space="PSUM"))

            loop_cm = tc.For_i(0, hw_loop, 1) if hw_loop else contextlib.nullcontext()
            with loop_cm:
              for it in range(niter):
                # The ENTIRE input (z chunks with interleaved mask slabs, plus
                # the W/rc/cnt/bias section riding the middle piece) streams
                # through a few dma_starts on the sync ring only: the HWDGE
                # descriptor generator is a single shared resource (~0.5us per
                # dma_start, serialized), so fewer+bigger pieces win.
                st = zpool.tile([128, SROW], zdt, name=f"st{it}", tag="st")
                pieces = list(zip(lay["bounds"][:-1], lay["bounds"][1:]))
                split = SPLITLAST and NG == 1 and len(pieces) >= 3
                order_pieces = (
                    [(pieces[0], nc.sync), (pieces[-1], nc.scalar)]
                    + [(p, nc.sync) for p in pieces[1:-1]]
                    if split
                    else [(p, nc.sync) for p in pieces]
                )
                for (a, bhi), eng in order_pieces:
                    eng.dma_start(
                        out=st[:, colbound(a) : colbound(bhi)],
                        in_=sb[:, colbound(a) : colbound(bhi)],
                    )
                w_t = st[:, lay["woff"] : lay["woff"] + DC * OUT * 2].bitcast(f16)
                rc_t = st[:, lay["rcoff"] : lay["rcoff"] + NG * NE * 4].bitcast(f32)
                cb_t = st[
                    0:1, lay["cboff"] : lay["cboff"] + (NG * NE + OUT) * 2
                ].bitcast(f16)
                cnt_t = cb_t[:, 0 : NG * NE]
                b_t = cb_t[:, NG * NE : NG * NE + OUT]

                for gi, g in enumerate(gorder):
                    T, C, CO = Ts[g], Cs[g], co[g]
                    last_g = gi == NG - 1
                    cb = {c: chunk_blocks[(g, c)] for c in range(C)}
                    # pool chunks in DATA-ARRIVAL order (final piece first
                    # when it rides the early scalar-ring DMA)
                    if split:
                        pool_order = (
                            list(range(*pieces[0]))
                            + list(range(*pieces[-1]))
                            + [c for a, b_ in pieces[1:-1] for c in range(a, b_)]
                        )
                    else:
                        pool_order = list(range(C))
                    pos = {c: i for i, c in enumerate(pool_order)}
                    lc = {}  # block -> emission position of its last chunk
                    for c in range(C):
                        for bidx, _ in cb[c]:
                            lc[bidx] = max(lc.get(bidx, -1), pos[c])

                    ps = psum_ps.tile([128, DC, NE], f32, name=f"ps{it}_{g}", tag="ps")
                    po = psum_po.tile([128, HO, NE], f32, name=f"po{it}_{g}", tag="po")
                    pt = ptpool.tile([128, DC, NE], f16, name=f"pt{it}_{g}", tag="pt")
                    ot = otpool.tile([128, HO, NE], f16, name=f"ot{it}_{g}", tag="ot")

                    def emit_copy(bidx, last_b):
                        # One DVE op; ACT is avoided entirely (any ACT op
                        # pulls a 1.3us activation-table load into the tail).
                        sl = slice(bidx * MW, (bidx + 1) * MW)
                        nc.vector.tensor_copy(pt[:, :, sl], ps[:, :, sl])

                    # PSUM start/stop semantics: start_tensor_calc pending-
                    # zeroes the ENTIRE 2KB bank (ZERO_REGION), and only one
                    # accumulation group may be open per bank. So each bank
                    # gets start=True on its very FIRST matmul and stop=True
                    # on its very LAST; every other write relies on the
                    # store-on-first-touch pending-zero semantics.
                    po_state = {"first": True}

                    def emit_linear(bidx):
                        sl = slice(bidx * MW, (bidx + 1) * MW)
                        cs = slice(g * NE + bidx * MW, g * NE + (bidx + 1) * MW)
                        # bias (x) counts seeds this block's po columns
                        for h in range(HO):
                            nc.tensor.matmul(
                                po[:, h, sl],
                                lhsT=b_t[0:1, h * 128 : (h + 1) * 128],
                                rhs=cnt_t[0:1, cs],
                                start=po_state["first"],
                                stop=False,
                            )
                            po_state["first"] = False
                        for h in range(HO):
                            for dc in range(DC):
                                nc.tensor.matmul(
                                    po[:, h, sl],
                                    lhsT=w_t[:, dc * OUT + h * 128 : dc * OUT + (h + 1) * 128],
                                    rhs=pt[:, dc, sl],
                                    start=False,
                                    stop=(
                                        bidx == NBLK - 1
                                        and h == HO - 1
                                        and dc == DC - 1
                                    ),
                                )

                    def emit_scale(bidx, last_b):
                        # All scales are emitted AFTER every copy: DVE is
                        # in-order, so a scale waiting on PE must never sit
                        # ahead of a later block's PSUM->SBUF copies.
                        sl = slice(bidx * MW, (bidx + 1) * MW)
                        cs = slice(g * NE + bidx * MW, g * NE + (bidx + 1) * MW)
                        rc_b = rc_t[:, cs].unsqueeze(1).broadcast_to([128, HO, MW])
                        nc.vector.tensor_mul(ot[:, :, sl], po[:, :, sl], rc_b)
                        if last_b:
                            # one out DMA for the whole group, on sync (its
                            # ring is idle once the input pieces are issued)
                            nc.sync.dma_start(out=out[g, :, :, :], in_=ot[:, :, :])

                    tail_due = {}
                    done_lin = set()
                    for bidx in range(NBLK - 1):  # last block's linear: post-loop
                        if lc[bidx] + LAG < C:
                            tail_due.setdefault(lc[bidx] + LAG, []).append(bidx)

                    ps_first = True
                    for ci, c in enumerate(pool_order):
                        zo = zoff(CO + c)
                        for blk in range(DC):
                            for i, (bidx, si) in enumerate(cb[c]):
                                nc.tensor.matmul(
                                    ps[:, blk, bidx * MW : (bidx + 1) * MW],
                                    lhsT=st[:, zo + blk * 128 : zo + (blk + 1) * 128],
                                    rhs=st[
                                        :, zo + D + si * MW : zo + D + (si + 1) * MW
                                    ],
                                    start=ps_first,
                                    stop=(
                                        ci == C - 1
                                        and blk == DC - 1
                                        and i == len(cb[c]) - 1
                                    ),
                                )
                                ps_first = False
                        for bidx in range(NBLK):
                            if lc[bidx] == ci:
                                emit_copy(bidx, last_b=(last_g and bidx == NBLK - 1))
                        for bidx in tail_due.get(ci, []):
                            emit_linear(bidx)
                            done_lin.add(bidx)
                    for bidx in range(NBLK):
                        if bidx not in done_lin:
                            emit_linear(bidx)
                    for bidx in range(NBLK):
                        emit_scale(bidx, last_b=(last_g and bidx == NBLK - 1))
    nc.finalize()
    return nc


def _span_info(sep):
    sep2 = np.asarray(sep).reshape(NSENT, K)
    starts = np.concatenate([np.ones_like(sep2[:, :1]), sep2[:, :-1] + 1], axis=-1)
    ends = sep2
    counts = (ends - starts).astype(np.float32)
    return sep2, starts, ends, counts


def _plan(sep):
    """Sort sentences by needed (span-member) rows; slot s gets global ranks
    [8s, 8s+8). Returns (order, caps)."""
    _, starts, ends, _ = _span_info(sep)
    R = np.clip(np.clip(ends - starts, 0, None).sum(-1), 1, L).astype(int)
    order = np.argsort(-R, kind="stable")
    caps = tuple(int(R[order[s * NCORES]]) for s in range(NSLOT))
    return order, caps


def _prep_in_maps(z, sep, Wf, bf, assign=None):
    order = assign if assign is not None else _plan(sep)[0]
    _, caps = _plan(sep)
    Ts, Cs, co, ctot, _ = _geom(caps)
    chunk_blocks = _blocks(caps)
    lay = _stream_layout(caps)
    RP, SROW, zoff = lay["rp"], lay["srow"], lay["zoff"]

    _, starts, ends, counts = _span_info(sep)       # [128, K]
    with np.errstate(divide="ignore"):
        rcounts = np.float32(1.0) / counts

    zdt = _z_np()
    zflat = z.reshape(NSENT, L, D)

    # Per-sentence span-row gather (+ entity label per row).
    idxs, labs = [], []
    for sid in range(NSENT):
        seg_i, seg_l = [], []
        for k in range(K):
            s_, e_ = int(starts[sid, k]), int(ends[sid, k])
            if e_ > s_:
                seg_i.append(np.arange(s_, e_))
                seg_l.append(np.full(e_ - s_, k))
        idxs.append(np.concatenate(seg_i) if seg_i else np.zeros(0, int))
        labs.append(np.concatenate(seg_l) if seg_l else np.zeros(0, int))
    cnt_rows = np.array([len(i) for i in idxs])
    maxc = max(int(cnt_rows.max()), 1)

    G = np.zeros((NSENT, maxc, D), np.float32)
    for sid in range(NSENT):
        G[sid, : cnt_rows[sid]] = zflat[sid, idxs[sid]]
    if DIFFUSE and zdt != np.float32:
        # Error-diffusion quantization along packed rows: each span-sum's
        # quantization error telescopes to ~1 quantum.
        Gq = np.empty((NSENT, maxc, D), zdt)
        carry = np.zeros((NSENT, D), np.float32)
        for l in range(maxc):
            v = G[:, l] + carry
            q = v.astype(zdt)
            Gq[:, l] = q
            carry = v - q.astype(np.float32)
    else:
        Gq = G.astype(zdt)

    # Section bytes shared by all cores: W (f16, [128, DC*OUT] d-on-partition),
    # rc (f32), cnt+b (f16, partition 0 only).
    Wm = Wf.astype(np.float16)  # [D, OUT]
    w_part = np.ascontiguousarray(
        Wm.reshape(DC, 128, OUT).transpose(1, 0, 2)
    ).view(np.uint8).reshape(128, DC * OUT * 2)
    bv = bf.astype(np.float16).reshape(OUT)

    in_maps = []
    for c in range(NCORES):
        stb = np.zeros((128, SROW), zdt)
        st8 = stb.view(np.uint8)
        rc = np.zeros((NG, NE), np.float32)
        cn = np.zeros((NG, NE), np.float16)
        for g in range(NG):
            T, C, CO = Ts[g], Cs[g], co[g]
            ztok = np.zeros((C * 128, D), zdt)
            mtok = np.zeros((C * 128, NE), zdt)
            off = 0
            for jslot in range(GS):
                s = g * GS + jslot
                sid = int(order[s * NCORES + c])
                cap = caps[s]
                n = min(int(cnt_rows[sid]), cap)
                ztok[off : off + n] = Gq[sid, :n]
                mtok[off + np.arange(n), jslot * K + labs[sid][:n]] = 1
                rc[g, jslot * K : (jslot + 1) * K] = rcounts[sid]
                cn[g, jslot * K : (jslot + 1) * K] = counts[sid]
                off += cap
            zch = ztok.reshape(C, 128, D).transpose(1, 0, 2)    # [128, C, D]
            mch = mtok.reshape(C, 128, NE).transpose(1, 0, 2)   # [128, C, NE]
            for cc in range(C):
                zo = zoff(CO + cc)
                stb[:, zo : zo + D] = zch[:, cc, :]
                for bidx, si in chunk_blocks[(g, cc)]:
                    stb[:, zo + D + si * MW : zo + D + (si + 1) * MW] = mch[
                        :, cc, bidx * MW : (bidx + 1) * MW
                    ]
        st8[:, lay["woff"] : lay["woff"] + DC * OUT * 2] = w_part
        rcb = np.broadcast_to(
            rc.reshape(1, NG * NE).view(np.uint8), (128, NG * NE * 4)
        )
        st8[:, lay["rcoff"] : lay["rcoff"] + NG * NE * 4] = rcb
        cbb = np.concatenate([cn.reshape(NG * NE), bv]).view(np.uint8)
        st8[0, lay["cboff"] : lay["cboff"] + (NG * NE + OUT) * 2] = cbb
        in_maps.append({"sb": stb})
    return in_maps


def _run(in_maps, nch_sj=None, **kwargs):
    from concourse.bass_utils import run_bass_kernel_spmd

    key = ("nc3", nch_sj, ZDT_MODE, MB, NG, ZBOUNDS, LAG, SPLITLAST)
    if key not in _CACHE:
        _CACHE[key] = _build_nc(nch_sj=nch_sj)
    return run_bass_kernel_spmd(_CACHE[key], in_maps, list(range(NCORES)), **kwargs)


def kernel(zipped_entity, entity_token_sep_idx, W, b):
    z = np.ascontiguousarray(np.asarray(zipped_entity, dtype=np.float32))
    sep = np.asarray(entity_token_sep_idx).astype(np.int64)
    Wf = np.ascontiguousarray(np.asarray(W, dtype=np.float32))
    bf = np.asarray(b, dtype=np.float32)
    assert z.shape == (BS, J, L, D) and sep.shape == (BS, J, K)

    order, caps = _plan(sep)
    res = _run(_prep_in_maps(z, sep, Wf, bf, assign=order), nch_sj=caps)
    out = np.empty((BS, J * K, OUT), np.float32)
    for c in range(NCORES):
        oc = res.results[c]["out"].astype(np.float32)  # [NG, 128, HO, NE]
        for s in range(NSLOT):
            g, jslot = divmod(s, GS)
            sid = int(order[s * NCORES + c])
            bb, jj = divmod(sid, J)
            for k in range(K):
                e = jslot * K + k
                out[bb, jj * K + k] = oc[g, :, :, e].T.reshape(OUT)
    return out


# revision 44
# speedup vs baseline: 1.0781x; 1.0004x over previous
"""EntityEncoder Trainium2 kernel (v3).

Computes, for each (batch, sentence j): ragged per-entity span mean-pool over
token embeddings, then a Linear projection:

    pooled[b, j, k, :] = mean(zipped_entity[b, j, start_kj:end_kj, :])
    out[b, j*K+k, :]   = pooled @ W + b

Strategy (8 NeuronCores, memory-bound):
  - Only span-member tokens are ever touched: rows outside the K spans
    (token 0 and the separator tokens themselves) are dropped at host-pack
    time. All 128 sentences are sorted by needed-row count and dealt
    rank-sliced: slot s (0..15) holds ranks [8s, 8s+8), one per core, so the
    shared NEFF's per-slot capacity (max of 8 nearly-equal counts) wastes ~2%.
    The NEFF is compile-time specialized on the 16 capacities (cached).
  - The z stream is fp8 e3m4 (4 mantissa bits; |z|<15.5 in range), quantized
    host-side with error diffusion along each sentence's packed rows so each
    span-sum's quantization error telescopes to ~1 quantum instead of
    sqrt(n) quanta; measured end-to-end rel err ~5e-3 (vs 4e-4 for f16) for
    half the HBM traffic.
  - ALL inputs ride ONE fused stream buffer in a handful of dma_starts on
    one ring: the HWDGE descriptor generator is a single shared resource
    (~0.5us serialized per dma_start, measured), concurrent rings/SWDGE
    degrade total DMA throughput ~20%, and each piece pays ~0.9us completion
    latency before its consumers wake. Each 128-token chunk record carries
    its own mask slabs inline (they arrive exactly with their data); W (f16),
    1/count (f32) and counts+bias (f16) live in a section spliced into the
    second piece via bitcast views. Piece boundaries: small first piece (PE
    starts early), a cut right after each non-final entity block's last
    chunk (its tail starts early), small final piece routed on the OTHER
    ring and POOLED FIRST, so the last-byte -> last-pool chain ends on an
    earlier, cheaper piece.
  - Pooling matmuls put Z as the STATIONARY operand and the 0/1 span masks
    as the moving operand: psum[d, e] += z_chunk[t, d]^T @ mask[t, e]. The
    pooled sums land already TRANSPOSED [d on partitions, entities free], so
    no tail transposes exist. PSUM start/stop: start_tensor_calc pending-
    zeroes the whole 2KB bank, so each bank gets start=True exactly once and
    stop=True on its final matmul; interior writes rely on store-on-first-
    touch semantics.
  - Per 32-entity block, when its last chunk is pooled: one DVE PSUM->SBUF
    f16 copy (never ACT: any ACT op drags a 1.3us activation-table load into
    the kernel), then a W-stationary Linear accumulating po[o, e], seeded by
    a rank-1 bias matmul b[o] (x) counts[e]. All 1/count scale-multiplies are
    emitted after every copy (the in-order DVE queue must never park a
    PE-dependent scale ahead of a later block's copies), and a single out
    DMA rides the idle sync ring at the end. (sums@W)*rc + b*(count*rc)
    reproduces the reference exactly, including 0/0 -> NaN (po=0, rc=inf ->
    NaN) and negative-count spans.
"""

import os
import numpy as np
from contextlib import ExitStack

BS, J, L, D = 32, 4, 512, 768
K = 4
OUT = 256
NCORES = 8
NSENT = BS * J           # 128 sentence tasks
NSLOT = NSENT // NCORES  # 16 slots per core
NG = int(os.environ.get("BASSK_NG", "1"))   # sentence groups per core
GS = NSLOT // NG         # slots (sentences) per group
NE = GS * K              # entities per group
DC = D // 128            # 6 D-chunks
HO = OUT // 128          # 2 output halves

# z/mask operand dtype: "f8e3" (default), "f8e4", "f16"
ZDT_MODE = os.environ.get("BASSK_ZDT", "f8e3")
# error-diffusion quantization of the z stream
DIFFUSE = os.environ.get("BASSK_DIFFUSE", "1") == "1"
# mask slab width (entity block granularity of the tail pipeline)
MB = int(os.environ.get("BASSK_MB", "32"))
MW = min(MB, NE)
NBLK = max(1, NE // MB)
# interior z piece boundaries (chunk units, scaled to ctot=16); "auto" =
# structural: small first piece, a cut right after each non-final block's
# last chunk, small final piece
ZBOUNDS = os.environ.get("BASSK_ZBOUNDS", "auto")
# chunks between a block's last pool matmul and its tail emission
LAG = int(os.environ.get("BASSK_LAG", "3"))
# issue the final piece early on the scalar ring and pool it first, so the
# stream's last-byte -> last-pool chain ends on an earlier piece
SPLITLAST = os.environ.get("BASSK_SPLITLAST", "1") == "1"

_CACHE = {}


def _z_dt(mybir):
    return {
        "f8e3": mybir.dt.float8e3,
        "f8e4": mybir.dt.float8e4,
        "f16": mybir.dt.float16,
    }[ZDT_MODE]


def _z_np():
    import ml_dtypes

    return {
        "f8e3": np.dtype(ml_dtypes.float8_e3m4),
        "f8e4": np.dtype(ml_dtypes.float8_e4m3),
        "f16": np.dtype(np.float16),
    }[ZDT_MODE]


def _geom(caps):
    """Per-group geometry from the 16 slot capacities: token totals, chunk
    counts, chunk offsets, group processing order (ascending size)."""
    caps = tuple(int(c) for c in caps)
    Ts = [sum(caps[g * GS : (g + 1) * GS]) for g in range(NG)]
    Cs = [(t + 127) // 128 for t in Ts]
    cum = np.cumsum([0] + Cs)
    co = [int(c) for c in cum[:-1]]
    ctot = int(cum[-1])
    gorder = sorted(range(NG), key=lambda g: Ts[g])
    return Ts, Cs, co, ctot, gorder


def _blocks(caps):
    """Entity-block coverage per (group, chunk): which MW-wide entity blocks
    the chunk's tokens touch, with the chunk-local slab slot of each."""
    Ts, Cs, co, ctot, _ = _geom(caps)
    chunk_blocks = {}            # (g, c) -> list of (block, local_slab_idx)
    for g in range(NG):
        offs = np.cumsum([0] + [caps[g * GS + j] for j in range(GS)])
        for c in range(Cs[g]):
            lo, hi = c * 128, min((c + 1) * 128, Ts[g])
            s0 = int(np.searchsorted(offs, lo, side="right")) - 1
            s1 = int(np.searchsorted(offs, hi - 1, side="right")) - 1
            blks = (
                sorted(set((K * s) // MW for s in range(s0, s1 + 1)))
                if NBLK > 1
                else [0]
            )
            chunk_blocks[(g, c)] = [(b, si) for si, b in enumerate(blks)]
    return chunk_blocks


def _piece_bounds(caps):
    """Chunk-piece boundaries of the fused stream (global chunk units). Each
    dma_start costs ~0.5us of serialized HWDGE descriptor generation, so few
    pieces; but a piece only becomes poolable when its WHOLE data lands
    (+~0.9us completion latency), so: small first piece (PE starts early), a
    cut right after each non-final entity block's last chunk (its tail chain
    starts as early as possible), small final piece."""
    Ts, Cs, co, ctot, _ = _geom(caps)
    cuts = {0, ctot}
    if ZBOUNDS != "auto":
        for b in ZBOUNDS.split(","):
            if b.strip():
                bi = (int(b) * ctot + 8) // 16  # scale the plan to this ctot
                if 0 < bi < ctot:
                    cuts.add(bi)
        return sorted(cuts)
    chunk_blocks = _blocks(caps)
    lc = {}
    for (g, c), lst in chunk_blocks.items():
        for bidx, _ in lst:
            lc[(g, bidx)] = max(lc.get((g, bidx), -1), co[g] + c)
    cuts.add(min(3, ctot))
    for (g, bidx), v in lc.items():
        if bidx < NBLK - 1 and 0 < v + 1 < ctot:
            cuts.add(v + 1)
    if ctot >= 4:
        cuts.add(ctot - 2)
    return sorted(cuts)


def _stream_layout(caps):
    """Fused single-stream layout: every input rides ONE dram buffer
    [128, SROW] of z-dtype bytes.

    Per partition row: ctot chunk records of RP bytes — 768 B of z followed
    by SLABMAX 32 B mask slabs (a chunk's masks arrive with its data) — with
    a section block (W as f16, rcount as f32, counts+bias as f16, bitcast
    views) spliced in before chunk SEC_CH so it rides the middle DMA piece.
    """
    Ts, Cs, co, ctot, _ = _geom(caps)
    chunk_blocks = _blocks(caps)
    slabmax = max(1, max(len(v) for v in chunk_blocks.values()))
    rp = D + slabmax * MW
    bounds = _piece_bounds(caps)
    sec_ch = bounds[1] if len(bounds) > 2 else 0
    secb = DC * OUT * 2 + NG * NE * 4 + (NG * NE + OUT) * 2
    sec0 = sec_ch * rp
    woff, rcoff, cboff = sec0, sec0 + DC * OUT * 2, sec0 + DC * OUT * 2 + NG * NE * 4
    srow = ctot * rp + secb

    def zoff(gc):
        return gc * rp + (secb if gc >= sec_ch else 0)

    def colbound(gc):
        return gc * rp + (secb if gc > sec_ch else 0)

    return {
        "rp": rp, "slabmax": slabmax, "bounds": bounds, "sec_ch": sec_ch,
        "secb": secb, "woff": woff, "rcoff": rcoff, "cboff": cboff,
        "srow": srow, "zoff": zoff, "colbound": colbound,
    }


def _build_nc(niter=1, hw_loop=0, nch_sj=None):
    """nch_sj: the 16-tuple of slot row capacities (compile-time plan)."""
    import contextlib

    import concourse.mybir as mybir
    from concourse.bacc import Bacc
    from concourse.tile import TileContext

    f32, f16 = mybir.dt.float32, mybir.dt.float16
    zdt = _z_dt(mybir)
    caps = nch_sj if nch_sj is not None else tuple(L for _ in range(NSLOT))
    Ts, Cs, co, ctot, gorder = _geom(caps)
    chunk_blocks = _blocks(caps)
    lay = _stream_layout(caps)
    RP, SROW, zoff, colbound = lay["rp"], lay["srow"], lay["zoff"], lay["colbound"]

    nc = Bacc(trn_type="TRN2")
    sb = nc.declare_dram_parameter("sb", [128, SROW], zdt, isOutput=False)
    out = nc.declare_dram_parameter("out", [NG, 128, HO, NE], f16, isOutput=True)

    with TileContext(nc) as tc:
        with ExitStack() as ctx:
            zpool = ctx.enter_context(tc.tile_pool(name="zp", bufs=min(niter, 2)))
            ptpool = ctx.enter_context(tc.tile_pool(name="pt", bufs=2))
            otpool = ctx.enter_context(tc.tile_pool(name="otp", bufs=2))
            psum_ps = ctx.enter_context(tc.tile_pool(name="ps", bufs=2, space="PSUM"))
            psum_po = ctx.enter_context(tc.tile_pool(name="po", bufs=2, space="PSUM"))

            loop_cm = tc.For_i(0, hw_loop, 1) if hw_loop else contextlib.nullcontext()
            with loop_cm:
              for it in range(niter):
                # The ENTIRE input (z chunks with interleaved mask slabs, plus
                # the W/rc/cnt/bias section riding the middle piece) streams
                # through a few dma_starts on the sync ring only: the HWDGE
                # descriptor generator is a single shared resource (~0.5us per
                # dma_start, serialized), so fewer+bigger pieces win.
                st = zpool.tile([128, SROW], zdt, name=f"st{it}", tag="st")
                pieces = list(zip(lay["bounds"][:-1], lay["bounds"][1:]))
                split = SPLITLAST and NG == 1 and len(pieces) >= 3
                order_pieces = (
                    [(pieces[0], nc.sync), (pieces[-1], nc.scalar)]
                    + [(p, nc.sync) for p in pieces[1:-1]]
                    if split
                    else [(p, nc.sync) for p in pieces]
                )
                for (a, bhi), eng in order_pieces:
                    eng.dma_start(
                        out=st[:, colbound(a) : colbound(bhi)],
                        in_=sb[:, colbound(a) : colbound(bhi)],
                    )
                w_t = st[:, lay["woff"] : lay["woff"] + DC * OUT * 2].bitcast(f16)
                rc_t = st[:, lay["rcoff"] : lay["rcoff"] + NG * NE * 4].bitcast(f32)
                cb_t = st[
                    0:1, lay["cboff"] : lay["cboff"] + (NG * NE + OUT) * 2
                ].bitcast(f16)
                cnt_t = cb_t[:, 0 : NG * NE]
                b_t = cb_t[:, NG * NE : NG * NE + OUT]

                for gi, g in enumerate(gorder):
                    T, C, CO = Ts[g], Cs[g], co[g]
                    last_g = gi == NG - 1
                    cb = {c: chunk_blocks[(g, c)] for c in range(C)}
                    # pool chunks in DATA-ARRIVAL order (final piece first
                    # when it rides the early scalar-ring DMA)
                    if split:
                        pool_order = (
                            list(range(*pieces[0]))
                            + list(range(*pieces[-1]))
                            + [c for a, b_ in pieces[1:-1] for c in range(a, b_)]
                        )
                    else:
                        pool_order = list(range(C))
                    pos = {c: i for i, c in enumerate(pool_order)}
                    lc = {}  # block -> emission position of its last chunk
                    for c in range(C):
                        for bidx, _ in cb[c]:
                            lc[bidx] = max(lc.get(bidx, -1), pos[c])

                    ps = psum_ps.tile([128, DC, NE], f32, name=f"ps{it}_{g}", tag="ps")
                    po = psum_po.tile([128, HO, NE], f32, name=f"po{it}_{g}", tag="po")
                    pt = ptpool.tile([128, DC, NE], f16, name=f"pt{it}_{g}", tag="pt")
                    ot = otpool.tile([128, HO, NE], f16, name=f"ot{it}_{g}", tag="ot")

                    def emit_copy(bidx, last_b):
                        # One DVE op; ACT is avoided entirely (any ACT op
                        # pulls a 1.3us activation-table load into the tail).
                        sl = slice(bidx * MW, (bidx + 1) * MW)
                        nc.vector.tensor_copy(pt[:, :, sl], ps[:, :, sl])

                    # PSUM start/stop semantics: start_tensor_calc pending-
                    # zeroes the ENTIRE 2KB bank (ZERO_REGION), and only one
                    # accumulation group may be open per bank. So each bank
                    # gets start=True on its very FIRST matmul and stop=True
                    # on its very LAST; every other write relies on the
                    # store-on-first-touch pending-zero semantics.
                    po_state = {"first": True}

                    def emit_linear(bidx):
                        sl = slice(bidx * MW, (bidx + 1) * MW)
                        cs = slice(g * NE + bidx * MW, g * NE + (bidx + 1) * MW)
                        # bias (x) counts seeds this block's po columns
                        for h in range(HO):
                            nc.tensor.matmul(
                                po[:, h, sl],
                                lhsT=b_t[0:1, h * 128 : (h + 1) * 128],
                                rhs=cnt_t[0:1, cs],
                                start=po_state["first"],
                                stop=False,
                            )
                            po_state["first"] = False
                        for h in range(HO):
                            for dc in range(DC):
                                nc.tensor.matmul(
                                    po[:, h, sl],
                                    lhsT=w_t[:, dc * OUT + h * 128 : dc * OUT + (h + 1) * 128],
                                    rhs=pt[:, dc, sl],
                                    start=False,
                                    stop=(
                                        bidx == NBLK - 1
                                        and h == HO - 1
                                        and dc == DC - 1
                                    ),
                                )

                    def emit_scale(bidx, last_b):
                        # All scales are emitted AFTER every copy: DVE is
                        # in-order, so a scale waiting on PE must never sit
                        # ahead of a later block's PSUM->SBUF copies.
                        sl = slice(bidx * MW, (bidx + 1) * MW)
                        cs = slice(g * NE + bidx * MW, g * NE + (bidx + 1) * MW)
                        rc_b = rc_t[:, cs].unsqueeze(1).broadcast_to([128, HO, MW])
                        nc.vector.tensor_mul(ot[:, :, sl], po[:, :, sl], rc_b)
                        if last_b:
                            # one out DMA for the whole group, on sync (its
                            # ring is idle once the input pieces are issued)
                            nc.sync.dma_start(out=out[g, :, :, :], in_=ot[:, :, :])

                    tail_due = {}
                    done_lin = set()
                    for bidx in range(NBLK - 1):  # last block's linear: post-loop
                        if lc[bidx] + LAG < C:
                            tail_due.setdefault(lc[bidx] + LAG, []).append(bidx)

                    ps_first = True
                    for ci, c in enumerate(pool_order):
                        zo = zoff(CO + c)
                        for blk in range(DC):
                            for i, (bidx, si) in enumerate(cb[c]):
                                nc.tensor.matmul(
                                    ps[:, blk, bidx * MW : (bidx + 1) * MW],
                                    lhsT=st[:, zo + blk * 128 : zo + (blk + 1) * 128],
                                    rhs=st[
                                        :, zo + D + si * MW : zo + D + (si + 1) * MW
                                    ],
                                    start=ps_first,
                                    stop=(
                                        ci == C - 1
                                        and blk == DC - 1
                                        and i == len(cb[c]) - 1
                                    ),
                                )
                                ps_first = False
                        for bidx in range(NBLK):
                            if lc[bidx] == ci:
                                emit_copy(bidx, last_b=(last_g and bidx == NBLK - 1))
                        for bidx in tail_due.get(ci, []):
                            emit_linear(bidx)
                            done_lin.add(bidx)
                    for bidx in range(NBLK):
                        if bidx not in done_lin:
                            emit_linear(bidx)
                    for bidx in range(NBLK):
                        emit_scale(bidx, last_b=(last_g and bidx == NBLK - 1))
    nc.finalize()
    return nc


def _span_info(sep):
    sep2 = np.asarray(sep).reshape(NSENT, K)
    starts = np.concatenate([np.ones_like(sep2[:, :1]), sep2[:, :-1] + 1], axis=-1)
    ends = sep2
    counts = (ends - starts).astype(np.float32)
    return sep2, starts, ends, counts


def _plan(sep):
    """Sort sentences by needed (span-member) rows; slot s gets global ranks
    [8s, 8s+8). Returns (order, caps)."""
    _, starts, ends, _ = _span_info(sep)
    R = np.clip(np.clip(ends - starts, 0, None).sum(-1), 1, L).astype(int)
    order = np.argsort(-R, kind="stable")
    caps = tuple(int(R[order[s * NCORES]]) for s in range(NSLOT))
    return order, caps


def _prep_in_maps(z, sep, Wf, bf, assign=None):
    order = assign if assign is not None else _plan(sep)[0]
    _, caps = _plan(sep)
    Ts, Cs, co, ctot, _ = _geom(caps)
    chunk_blocks = _blocks(caps)
    lay = _stream_layout(caps)
    RP, SROW, zoff = lay["rp"], lay["srow"], lay["zoff"]

    _, starts, ends, counts = _span_info(sep)       # [128, K]
    with np.errstate(divide="ignore"):
        rcounts = np.float32(1.0) / counts

    zdt = _z_np()
    zflat = z.reshape(NSENT, L, D)

    # Per-sentence span-row gather (+ entity label per row).
    idxs, labs = [], []
    for sid in range(NSENT):
        seg_i, seg_l = [], []
        for k in range(K):
            s_, e_ = int(starts[sid, k]), int(ends[sid, k])
            if e_ > s_:
                seg_i.append(np.arange(s_, e_))
                seg_l.append(np.full(e_ - s_, k))
        idxs.append(np.concatenate(seg_i) if seg_i else np.zeros(0, int))
        labs.append(np.concatenate(seg_l) if seg_l else np.zeros(0, int))
    cnt_rows = np.array([len(i) for i in idxs])
    maxc = max(int(cnt_rows.max()), 1)

    G = np.zeros((NSENT, maxc, D), np.float32)
    for sid in range(NSENT):
        G[sid, : cnt_rows[sid]] = zflat[sid, idxs[sid]]
    if DIFFUSE and zdt != np.float32:
        # Error-diffusion quantization along packed rows: each span-sum's
        # quantization error telescopes to ~1 quantum.
        Gq = np.empty((NSENT, maxc, D), zdt)
        carry = np.zeros((NSENT, D), np.float32)
        for l in range(maxc):
            v = G[:, l] + carry
            q = v.astype(zdt)
            Gq[:, l] = q
            carry = v - q.astype(np.float32)
    else:
        Gq = G.astype(zdt)

    # Section bytes shared by all cores: W (f16, [128, DC*OUT] d-on-partition),
    # rc (f32), cnt+b (f16, partition 0 only).
    Wm = Wf.astype(np.float16)  # [D, OUT]
    w_part = np.ascontiguousarray(
        Wm.reshape(DC, 128, OUT).transpose(1, 0, 2)
    ).view(np.uint8).reshape(128, DC * OUT * 2)
    bv = bf.astype(np.float16).reshape(OUT)

    in_maps = []
    for c in range(NCORES):
        stb = np.zeros((128, SROW), zdt)
        st8 = stb.view(np.uint8)
        rc = np.zeros((NG, NE), np.float32)
        cn = np.zeros((NG, NE), np.float16)
        for g in range(NG):
            T, C, CO = Ts[g], Cs[g], co[g]
            ztok = np.zeros((C * 128, D), zdt)
            mtok = np.zeros((C * 128, NE), zdt)
            off = 0
            for jslot in range(GS):
                s = g * GS + jslot
                sid = int(order[s * NCORES + c])
                cap = caps[s]
                n = min(int(cnt_rows[sid]), cap)
                ztok[off : off + n] = Gq[sid, :n]
                mtok[off + np.arange(n), jslot * K + labs[sid][:n]] = 1
                rc[g, jslot * K : (jslot + 1) * K] = rcounts[sid]
                cn[g, jslot * K : (jslot + 1) * K] = counts[sid]
                off += cap
            zch = ztok.reshape(C, 128, D).transpose(1, 0, 2)    # [128, C, D]
            mch = mtok.reshape(C, 128, NE).transpose(1, 0, 2)   # [128, C, NE]
            for cc in range(C):
                zo = zoff(CO + cc)
                stb[:, zo : zo + D] = zch[:, cc, :]
                for bidx, si in chunk_blocks[(g, cc)]:
                    stb[:, zo + D + si * MW : zo + D + (si + 1) * MW] = mch[
                        :, cc, bidx * MW : (bidx + 1) * MW
                    ]
        st8[:, lay["woff"] : lay["woff"] + DC * OUT * 2] = w_part
        rcb = np.broadcast_to(
            rc.reshape(1, NG * NE).view(np.uint8), (128, NG * NE * 4)
        )
        st8[:, lay["rcoff"] : lay["rcoff"] + NG * NE * 4] = rcb
        cbb = np.concatenate([cn.reshape(NG * NE), bv]).view(np.uint8)
        st8[0, lay["cboff"] : lay["cboff"] + (NG * NE + OUT) * 2] = cbb
        in_maps.append({"sb": stb})
    return in_maps


def _run(in_maps, nch_sj=None, **kwargs):
    from concourse.bass_utils import run_bass_kernel_spmd

    key = ("nc3", nch_sj, ZDT_MODE, MB, NG, ZBOUNDS, LAG, SPLITLAST)
    if key not in _CACHE:
        _CACHE[key] = _build_nc(nch_sj=nch_sj)
    return run_bass_kernel_spmd(_CACHE[key], in_maps, list(range(NCORES)), **kwargs)


def kernel(zipped_entity, entity_token_sep_idx, W, b):
    z = np.ascontiguousarray(np.asarray(zipped_entity, dtype=np.float32))
    sep = np.asarray(entity_token_sep_idx).astype(np.int64)
    Wf = np.ascontiguousarray(np.asarray(W, dtype=np.float32))
    bf = np.asarray(b, dtype=np.float32)
    assert z.shape == (BS, J, L, D) and sep.shape == (BS, J, K)

    order, caps = _plan(sep)
    res = _run(_prep_in_maps(z, sep, Wf, bf, assign=order), nch_sj=caps)
    out = np.empty((BS, J * K, OUT), np.float32)
    for c in range(NCORES):
        oc = res.results[c]["out"].astype(np.float32)  # [NG, 128, HO, NE]
        for s in range(NSLOT):
            g, jslot = divmod(s, GS)
            sid = int(order[s * NCORES + c])
            bb, jj = divmod(sid, J)
            for k in range(K):
                e = jslot * K + k
                out[bb, jj * K + k] = oc[g, :, :, e].T.reshape(OUT)
    return out
